# revision 1
# baseline (speedup 1.0000x reference)
"""Chamfer L1 distance kernel for Trainium2 (8 NeuronCores).

Full inputs: pred [4, 8192, 3] f32, target [4, 8192, 3] f32.
Output: scalar f32 = mean over batch of (sum_i min_j d(i,j) + sum_j min_i d(i,j)),
d = L1 distance.

Sharding: 8 cores = 4 batches x 2 pred-halves. Each core handles its 4096 preds
vs all 8192 targets and outputs:
  - rowmin [128, 32] bf16: rowmin[p, b] = min over all j of dist for pred
    (block b, partition p)
  - colmin [128, 8192] bf16: colmin[p, j] = min over this core's pred blocks
    (partition-p lane) of dist
Host finishes the reductions (min over partitions/core-pairs, sums, /B).

Device pipeline per 128-pred block b (j in 4096-wide act-chunks, DVE ops on
2048-wide slices):
  A_d = |T_d - p_d|   scalar.activation(Abs, bias=-p_d)  f32 -> bf16   (ACT x3)
  S01 = A0 + A1       vector.tensor_tensor add  bf16 2x               (DVE)
  S   = S01 + A2      vector.tensor_tensor add  bf16 2x               (DVE)
  rowacc = min(rowacc, S) across chunks; one tensor_reduce per block  (DVE)
  colmin chunk = min(colmin, S)                                       (DVE)
T_d are the target coords broadcast across all 128 partitions with a single
stride-0 broadcast DMA each. Intermediates are bf16 (end-to-end rel err vs the
fp32 reference ~3e-5); inputs stay f32 so no precision is lost in t - p.
"""

import sys

sys.path.insert(0, "/opt/trn_rl_repo")

import numpy as np

N_CORES = 8
B, N, M = 4, 8192, 8192
P = 128
NPRED = N // 2  # preds per core
NBLK = NPRED // P  # 32
CH = 2048  # j-chunk size
NCH = M // CH

_compiled = None


def _build(reps=1, wbufs=4, act_ch=4096, dve_ch=CH, abufs_override=4):
    import concourse.bacc as bacc
    import concourse.mybir as mybir
    import concourse.tile as tile

    f32 = mybir.dt.float32
    bf16 = mybir.dt.bfloat16
    Alu = mybir.AluOpType
    Act = mybir.ActivationFunctionType

    nc = bacc.Bacc("TRN2", debug=False, num_devices=N_CORES)
    pred_rn = nc.dram_tensor("pred_rn", [P, NBLK * 3], f32, kind="ExternalInput").ap()
    target_t = nc.dram_tensor("target_t", [3, M], f32, kind="ExternalInput").ap()
    rowmin_d = nc.dram_tensor("rowmin", [P, NBLK], bf16, kind="ExternalOutput").ap()
    colmin_d = nc.dram_tensor("colmin", [P, M], bf16, kind="ExternalOutput").ap()

    BIG = 3.0e38

    with tile.TileContext(nc) as tc:
        abufs = 2 if act_ch > CH else abufs_override
        with (
            tc.tile_pool(name="const", bufs=1) as cpool,
            tc.tile_pool(name="work", bufs=wbufs) as wpool,
            tc.tile_pool(name="apool", bufs=abufs) as apool,
        ):
            PNt = cpool.tile([P, NBLK * 3], f32, tag="PN")
            nc.sync.dma_start(PNt[:, :], pred_rn[:, :])

            T = [cpool.tile([P, M], f32, tag=f"T{d}", name=f"T{d}") for d in range(3)]
            for d in range(3):
                nc.sync.dma_start(
                    T[d][:, :], target_t[d : d + 1, :].broadcast_to([P, M])
                )

            colmin = cpool.tile([P, M], bf16, tag="colmin")
            nc.vector.memset(colmin[:, :], BIG)
            rowmin = cpool.tile([P, NBLK], bf16, tag="rowmin")
            rowacc = cpool.tile([P, CH], bf16, tag="rowacc")
            rowacc2 = cpool.tile([P, CH], bf16, tag="rowacc2")

            import contextlib

            loop_ctx = tc.For_i(0, reps, 1) if reps > 1 else contextlib.nullcontext()
            with loop_ctx:
              for b in range(NBLK):
                n0 = PNt[:, 3 * b : 3 * b + 1]
                n1 = PNt[:, 3 * b + 1 : 3 * b + 2]
                n2 = PNt[:, 3 * b + 2 : 3 * b + 3]
                for ac in range(M // act_ch):
                    ajs = slice(ac * act_ch, (ac + 1) * act_ch)
                    A0 = apool.tile([P, act_ch], bf16, tag="A0")
                    nc.scalar.activation(A0[:, :], T[0][:, ajs], Act.Abs, bias=n0, scale=1.0)
                    A1 = apool.tile([P, act_ch], bf16, tag="A1")
                    nc.scalar.activation(A1[:, :], T[1][:, ajs], Act.Abs, bias=n1, scale=1.0)
                    A2 = apool.tile([P, act_ch], bf16, tag="A2")
                    nc.scalar.activation(A2[:, :], T[2][:, ajs], Act.Abs, bias=n2, scale=1.0)
                    for c2 in range(act_ch // dve_ch):
                        c = ac * (act_ch // dve_ch) + c2
                        js = slice(
                            ac * act_ch + c2 * dve_ch, ac * act_ch + (c2 + 1) * dve_ch
                        )
                        cs = slice(c2 * dve_ch, (c2 + 1) * dve_ch)
                        S01 = wpool.tile([P, dve_ch], bf16, tag="S01")
                        nc.vector.tensor_tensor(S01[:, :], A0[:, cs], A1[:, cs], Alu.add)
                        S = wpool.tile([P, dve_ch], bf16, tag="S")
                        nc.vector.tensor_tensor(S[:, :], S01[:, :], A2[:, cs], Alu.add)
                        for h in range(dve_ch // CH):
                            hs = slice(h * CH, (h + 1) * CH)
                            k = c * (dve_ch // CH) + h
                            dst = rowacc if k % 2 == 0 else rowacc2
                            src = rowacc2 if k % 2 == 0 else rowacc
                            if k == 0:
                                nc.vector.tensor_copy(dst[:, :], S[:, hs])
                            else:
                                nc.vector.tensor_tensor(
                                    dst[:, :], src[:, :], S[:, hs], Alu.min
                                )
                        if c == M // dve_ch - 1:
                            nc.vector.tensor_reduce(
                                rowmin[:, b : b + 1],
                                dst[:, :],
                                mybir.AxisListType.X,
                                Alu.min,
                            )
                        nc.vector.tensor_tensor(
                            colmin[:, js], colmin[:, js], S[:, :], Alu.min
                        )

            nc.sync.dma_start(rowmin_d[:, :], rowmin[:, :])
            nc.sync.dma_start(colmin_d[:, :], colmin[:, :])

    nc.compile()
    return nc


def _shard(pred, target):
    in_maps = []
    for c in range(N_CORES):
        b, h = c // 2, c % 2
        pr = pred[b, h * NPRED : (h + 1) * NPRED, :]  # [4096, 3]
        prn = np.ascontiguousarray(
            -pr.reshape(NBLK, P, 3).transpose(1, 0, 2).reshape(P, NBLK * 3)
        )
        tt = np.ascontiguousarray(target[b].T)  # [3, 8192]
        in_maps.append({"pred_rn": prn, "target_t": tt})
    return in_maps


def _combine(results):
    total = 0.0
    for b in range(B):
        bwd = None
        for r in (results[2 * b], results[2 * b + 1]):
            rm = np.asarray(r["rowmin"]).astype(np.float32)  # [128, 32]
            total += float(rm.sum(dtype=np.float64))
            cm = np.asarray(r["colmin"]).astype(np.float32).min(axis=0)  # [8192]
            bwd = cm if bwd is None else np.minimum(bwd, cm)
        total += float(bwd.sum(dtype=np.float64))
    return np.float32(total / B)


def kernel(pred, target):
    global _compiled
    from concourse import bass_utils

    pred = np.asarray(pred, dtype=np.float32)
    target = np.asarray(target, dtype=np.float32)
    if _compiled is None:
        _compiled = _build()
    in_maps = _shard(pred, target)
    res = bass_utils.run_bass_kernel_spmd(
        _compiled, in_maps, core_ids=list(range(N_CORES))
    )
    return _combine(res.results)



# revision 3
# speedup vs baseline: 10.2934x; 10.2934x over previous
"""Chamfer L1 distance kernel for Trainium2 (8 NeuronCores) — sorted-window
algorithm.

Full inputs: pred [4, 8192, 3] f32, target [4, 8192, 3] f32.
Output: scalar f32 = mean over batch of (sum_i min_j d(i,j) + sum_j min_i d(i,j)),
d = L1 distance.

Algorithm (exact, with on-host verification + fallback):
  d(p,t) = |dx|+|dy|+|dz| >= |u_p - u_t| with u = x+y+z.  Sort preds and
  targets of each batch by u.  A pred at sorted rank g only needs to scan
  targets in a rank window centered at g (counts match, so ranks align by
  quantile); any target outside the window is at u-distance >= the window
  edge gap.  After the device pass, the host checks every returned min m
  against its window-edge u-gap; the ~0.4% of points whose NN might lie
  outside their window (locally sparse regions) are recomputed exactly on
  host.  Device mins are upper bounds, so the check is sound.

Sharding: 8 cores = 4 batches x 2 pred-halves (sorted rank split).  Each core:
32 blocks of 128 preds x K-wide target window (window slides 128 ranks per
block).  Device outputs per core:
  - rowmin [128, 32] bf16: min over the pred's window
  - colmin [128, W] bf16 (W = 4096+K-128): running min over this core's pred
    blocks for each covered target rank (partition p = pred lane; host
    reduces over partitions/cores)
Engine split per block b (f32 T tiles resident in SBUF, bf16 intermediates):
  ACT: A_d = |T_d - p_d|  (Abs activation, bias=-p_d; subtract in f32 then
       round to bf16 — no cancellation); on odd blocks |Dz| moves to DVE
       (raw diff + negate + max) to balance engine load.
  DVE: S01 = A0+A1, S = S01+A2 (TT add 2x), colmin = min(colmin, S),
       rowmin: fold K->K/2->K/4 (TT min) then tensor_reduce.
"""

import sys

sys.path.insert(0, "/opt/trn_rl_repo")

import numpy as np

N_CORES = 8
B, N, M = 4, 8192, 8192
P = 128
NPRED = N // 2  # preds per core
NBLK = NPRED // P  # 32
K = 512  # target window width (ranks)
W = NPRED + K - P  # colmin/target tile width per core
SENTINEL = 30000.0
BIG = 60000.0
ALT_Z = 2  # every ALT_Z-th block computes |Dz| on DVE instead of ACT (0 = never)

_compiled = None


def _build(reps=1):
    import concourse.bacc as bacc
    import concourse.mybir as mybir
    import concourse.tile as tile

    f32 = mybir.dt.float32
    bf16 = mybir.dt.bfloat16
    Alu = mybir.AluOpType
    Act = mybir.ActivationFunctionType

    nc = bacc.Bacc("TRN2", debug=False, num_devices=N_CORES)
    pred_rn = nc.dram_tensor("pred_rn", [P, NBLK * 3], f32, kind="ExternalInput").ap()
    target_t = nc.dram_tensor("target_t", [3, W], f32, kind="ExternalInput").ap()
    rowmin_d = nc.dram_tensor("rowmin", [P, NBLK], bf16, kind="ExternalOutput").ap()
    colmin_d = nc.dram_tensor("colmin", [P, W], bf16, kind="ExternalOutput").ap()

    with tile.TileContext(nc) as tc:
        with (
            tc.tile_pool(name="const", bufs=1) as cpool,
            tc.tile_pool(name="apool", bufs=4) as apool,
            tc.tile_pool(name="wpool", bufs=4) as wpool,
        ):
            PNt = cpool.tile([P, NBLK * 3], f32, tag="PN")
            nc.sync.dma_start(PNt[:, :], pred_rn[:, :])

            T = [cpool.tile([P, W], f32, tag=f"T{d}", name=f"T{d}") for d in range(3)]
            for d in range(3):
                nc.sync.dma_start(
                    T[d][:, :], target_t[d : d + 1, :].broadcast_to([P, W])
                )

            colmin = cpool.tile([P, W], bf16, tag="colmin")
            nc.vector.memset(colmin[:, :], BIG)
            rowmin = cpool.tile([P, NBLK], bf16, tag="rowmin")

            import contextlib

            loop_ctx = tc.For_i(0, reps, 1) if reps > 1 else contextlib.nullcontext()
            with loop_ctx:
                for r in range(NBLK):
                    ws = slice(P * r, P * r + K)
                    biases = [PNt[:, 3 * r + d : 3 * r + d + 1] for d in range(3)]
                    z_on_dve = ALT_Z > 0 and (r % ALT_Z == ALT_Z - 1)
                    A = []
                    for d in range(2):
                        Ad = apool.tile([P, K], bf16, tag=f"A{d}", name=f"A{d}")
                        nc.scalar.activation(
                            Ad[:, :], T[d][:, ws], Act.Abs, bias=biases[d], scale=1.0
                        )
                        A.append(Ad)
                    A2 = apool.tile([P, K], bf16, tag="A2")
                    if z_on_dve:
                        Dz = wpool.tile([P, K], bf16, tag="Dz")
                        nc.vector.tensor_scalar_add(Dz[:, :], T[2][:, ws], biases[2])
                        nDz = wpool.tile([P, K], bf16, tag="nDz")
                        nc.vector.tensor_scalar_mul(nDz[:, :], Dz[:, :], -1.0)
                        nc.vector.tensor_tensor(A2[:, :], Dz[:, :], nDz[:, :], Alu.max)
                    else:
                        nc.scalar.activation(
                            A2[:, :], T[2][:, ws], Act.Abs, bias=biases[2], scale=1.0
                        )
                    S01 = wpool.tile([P, K], bf16, tag="S01")
                    nc.vector.tensor_tensor(S01[:, :], A[0][:, :], A[1][:, :], Alu.add)
                    S = wpool.tile([P, K], bf16, tag="S")
                    nc.vector.tensor_tensor(S[:, :], S01[:, :], A2[:, :], Alu.add)
                    nc.vector.tensor_tensor(
                        colmin[:, ws], colmin[:, ws], S[:, :], Alu.min
                    )
                    F1 = wpool.tile([P, K // 2], bf16, tag="F1")
                    nc.vector.tensor_tensor(
                        F1[:, :], S[:, : K // 2], S[:, K // 2 :], Alu.min
                    )
                    F2 = wpool.tile([P, K // 4], bf16, tag="F2")
                    nc.vector.tensor_tensor(
                        F2[:, :], F1[:, : K // 4], F1[:, K // 4 :], Alu.min
                    )
                    nc.vector.tensor_reduce(
                        rowmin[:, r : r + 1], F2[:, :], mybir.AxisListType.X, Alu.min
                    )

            nc.sync.dma_start(rowmin_d[:, :], rowmin[:, :])
            nc.sync.dma_start(colmin_d[:, :], colmin[:, :])

    nc.compile()
    return nc


def _sort_batch(pred_b, target_b):
    up = pred_b.sum(1)
    ut = target_b.sum(1)
    po = np.argsort(up, kind="stable")
    to = np.argsort(ut, kind="stable")
    return pred_b[po], target_b[to], up[po], ut[to]


def _shard(pred, target):
    HALF = K // 2
    in_maps = []
    meta = []
    for b in range(B):
        ps, ts, ups, uts = _sort_batch(pred[b], target[b])
        meta.append((ps, ts, ups, uts))
        for h in range(2):
            pr = ps[h * NPRED : (h + 1) * NPRED]  # [4096, 3]
            prn = np.ascontiguousarray(
                -pr.reshape(NBLK, P, 3).transpose(1, 0, 2).reshape(P, NBLK * 3)
            )
            G0 = NPRED * h + P // 2 - HALF
            Tpad = np.full((W, 3), SENTINEL, np.float32)
            lo, hi = max(0, G0), min(M, G0 + W)
            Tpad[lo - G0 : hi - G0] = ts[lo:hi]
            tt = np.ascontiguousarray(Tpad.T)  # [3, W]
            in_maps.append({"pred_rn": prn, "target_t": tt})
    return in_maps, meta


def _combine(results, meta):
    HALF = K // 2
    total = 0.0
    for b in range(B):
        ps, ts, ups, uts = meta[b]
        m_row = np.full(N, np.inf, np.float32)
        m_col = np.full(M, np.inf, np.float32)
        covA = np.full(M, N, np.int64)
        covB = np.full(M, -1, np.int64)
        for h in range(2):
            r = results[2 * b + h]
            rm = np.asarray(r["rowmin"]).astype(np.float32)  # [128, 32]
            # rowmin[p, blk] = pred at sorted rank NPRED*h + 128*blk + p
            gidx = NPRED * h + P * np.arange(NBLK)[None, :] + np.arange(P)[:, None]
            m_row[gidx.ravel()] = rm.ravel()
            cm = np.asarray(r["colmin"]).astype(np.float32).min(axis=0)  # [W]
            G0 = NPRED * h + P // 2 - HALF
            gt = G0 + np.arange(W)
            valid = (gt >= 0) & (gt < M)
            np.minimum.at(m_col, gt[valid], cm[valid])
            # coverage: block blk covers targets [G0+128*blk, G0+128*blk+K)
            for blk in range(NBLK):
                g = G0 + P * blk + np.arange(K)
                v = (g >= 0) & (g < M)
                covA[g[v]] = np.minimum(covA[g[v]], NPRED * h + P * blk)
                covB[g[v]] = np.maximum(covB[g[v]], NPRED * h + P * blk + P)
        # verification: rowmin
        h_arr = np.arange(N) // NPRED
        r_arr = (np.arange(N) % NPRED) // P
        wlo = NPRED * h_arr + P * r_arr + P // 2 - HALF
        whi = wlo + K
        gap_lo = np.where(wlo > 0, ups - uts[np.clip(wlo, 1, M) - 1], np.inf)
        gap_hi = np.where(whi < M, uts[np.clip(whi, 0, M - 1)] - ups, np.inf)
        ok_r = m_row <= np.minimum(gap_lo, gap_hi)
        for g in np.where(~ok_r)[0]:
            m_row[g] = np.abs(ps[g][None, :] - ts).sum(1).min()
        # verification: colmin
        gap_lo_c = np.where(covA > 0, uts - ups[np.clip(covA, 1, N) - 1], np.inf)
        gap_hi_c = np.where(covB < N, ups[np.clip(covB, 0, N - 1)] - uts, np.inf)
        ok_c = (m_col <= np.minimum(gap_lo_c, gap_hi_c)) & (covB > covA)
        for j in np.where(~ok_c)[0]:
            m_col[j] = np.abs(ts[j][None, :] - ps).sum(1).min()
        total += m_row.sum(dtype=np.float64) + m_col.sum(dtype=np.float64)
    return np.float32(total / B)


def kernel(pred, target):
    global _compiled
    from concourse import bass_utils

    pred = np.asarray(pred, dtype=np.float32)
    target = np.asarray(target, dtype=np.float32)
    if _compiled is None:
        _compiled = _build()
    in_maps, meta = _shard(pred, target)
    res = bass_utils.run_bass_kernel_spmd(
        _compiled, in_maps, core_ids=list(range(N_CORES))
    )
    return _combine(res.results, meta)
